# revision 3
# baseline (speedup 1.0000x reference)
"""Distributed Bass attention kernel for 8 TRN2 NeuronCores.

Problem: single-head causal attention, B=4, S=2048, d_model=1024, d_head=64.
  q = x@WQ.T+bq; k = x@WK.T+bk; v = x@WV.T+bv (v is d_model wide)
  out = softmax(causal(q@k.T)) @ v

Sharding: core = 2*b + half. Each core computes batch b, output channels
[half*512, (half+1)*512). Q/K/scores/softmax are duplicated within a batch
pair (cheap); V projection and attn@V are channel-split. No collectives.

Layout tricks:
  - x fed pre-transposed (xT [d, S]) so projections contract d on partitions.
  - scores computed transposed [keys, queries] (kT stationary, qT moving) so
    attn@V uses P tiles directly as the stationary operand - no transposes.
  - softmax without max-subtraction (|logits| <= ~45 => exp fits fp32 fine);
    rowsum via an extra N=1 matmul vs a ones vector; normalize at the end
    with DVE reciprocal + per-partition scalar multiply.
  - float32r (1 PE cycle/row vs 4 for fp32) for projection/score matmuls;
    bf16 for P/V in attn@V (post-softmax quantization, ~0.4% weight noise).
"""

import sys

if "/opt/trn_rl_repo" not in sys.path:
    sys.path.insert(0, "/opt/trn_rl_repo")

import numpy as np

from concourse import bacc, tile, mybir
import concourse.bass as bass
from concourse.bass_utils import run_bass_kernel_spmd

B, S, D, HD = 4, 2048, 1024, 64
N_CORES = 8
CPC = 512  # output channels per core
NCHUNK = 8  # d_model / 128

f32 = mybir.dt.float32
f32r = mybir.dt.float32r
bf16 = mybir.dt.bfloat16
AF = mybir.ActivationFunctionType
ALU = mybir.AluOpType

_cache = {}


def _build():
    nc = bacc.Bacc("TRN2", target_bir_lowering=False, debug=False, num_devices=N_CORES)

    xT = nc.dram_tensor("xT", [NCHUNK, 128, S], f32r, kind="ExternalInput")
    wqkT = nc.dram_tensor("wqkT", [NCHUNK, 128, 128], f32r, kind="ExternalInput")
    bqk = nc.dram_tensor("bqk", [128, 1], f32, kind="ExternalInput")
    wvT = nc.dram_tensor("wvT", [NCHUNK, 128, CPC], f32r, kind="ExternalInput")
    bv = nc.dram_tensor("bv", [1, CPC], f32r, kind="ExternalInput")
    masks = nc.dram_tensor("masks", [4, 128, 512], bf16, kind="ExternalInput")
    ones1 = nc.dram_tensor("ones1", [1, 128], f32r, kind="ExternalInput")
    out = nc.dram_tensor("out", [16, 128, CPC], f32, kind="ExternalOutput")

    def r(ap):
        return ap.bitcast(f32r)

    with tile.TileContext(nc) as tc:
        with (
            tc.tile_pool(name="big", bufs=1) as big,
            tc.tile_pool(name="ppool", bufs=20) as ppool,
            tc.tile_pool(name="opool", bufs=3) as opool,
            tc.tile_pool(name="small", bufs=4) as small,
            tc.tile_pool(name="ps_s", bufs=2, space=bass.MemorySpace.PSUM) as ps_s,
            tc.tile_pool(name="ps_v", bufs=2, space=bass.MemorySpace.PSUM) as ps_v,
            tc.tile_pool(name="ps_o", bufs=2, space=bass.MemorySpace.PSUM) as ps_o,
            tc.tile_pool(name="ps_r", bufs=2, space=bass.MemorySpace.PSUM) as ps_r,
        ):
            # persistent SBUF tiles
            xt = big.tile([128, NCHUNK, S], f32r, tag="xt")  # 64KB/p
            wqk = big.tile([128, NCHUNK, 128], f32r, tag="wqk")  # 4KB/p
            wv = big.tile([128, NCHUNK, CPC], f32r, tag="wv")  # 16KB/p
            bqk_sb = big.tile([128, 1], f32, tag="bqk")
            bv_sb = big.tile([1, CPC], f32r, tag="bv")
            mask_sb = big.tile([128, 4, 512], bf16, tag="mask")  # 4KB/p
            qk_sb = big.tile([128, S], f32r, tag="qk")  # 8KB/p
            kt_sb = big.tile([64, S], f32r, tag="kt")  # 8KB/p
            v_sb = big.tile([128, 16, CPC], bf16, tag="v")  # 16KB/p
            ones_k = big.tile([128, 1], bf16, tag="ones_k")
            ones_1 = big.tile([1, 128], f32r, tag="ones_1")

            for c in range(NCHUNK):
                nc.sync.dma_start(out=xt[:, c, :], in_=xT[c, :, :])
                nc.sync.dma_start(out=wqk[:, c, :], in_=wqkT[c, :, :])
                nc.sync.dma_start(out=wv[:, c, :], in_=wvT[c, :, :])
            nc.sync.dma_start(out=bqk_sb[:, :], in_=bqk[:, :])
            nc.sync.dma_start(out=bv_sb[:, :], in_=bv[:, :])
            for m in range(4):
                nc.sync.dma_start(out=mask_sb[:, m, :], in_=masks[m, :, :])
            nc.vector.memset(ones_k[:, :], 1.0)
            nc.sync.dma_start(out=ones_1[:, :], in_=ones1[:, :])

            # ---- Q/K projection: qkT [128h (64 q + 64 k), S] ----
            for j in range(4):
                qk_ps = ps_s.tile([128, 512], f32, tag="scps")
                for c in range(NCHUNK):
                    nc.tensor.matmul(
                        qk_ps[:, :],
                        wqk[:, c, :],
                        xt[:, c, 512 * j : 512 * (j + 1)],
                        start=(c == 0),
                        stop=(c == NCHUNK - 1),
                    )
                nc.scalar.activation(
                    qk_sb[:, 512 * j : 512 * (j + 1)],
                    qk_ps[:, :],
                    AF.Identity,
                    bias=bqk_sb[:, 0:1],
                )
            # kT rows (64..128) -> partitions 0..64 via SBUF->SBUF DMA
            nc.sync.dma_start(out=kt_sb[:, :], in_=qk_sb[64:128, :])

            # ---- V projection (with bias folded via K=1 ones matmul) ----
            for t in range(16):
                v_ps = ps_v.tile([128, CPC], f32, tag="vps")
                for c in range(NCHUNK):
                    nc.tensor.matmul(
                        v_ps[:, :],
                        xt[:, c, 128 * t : 128 * (t + 1)],
                        wv[:, c, :],
                        start=(c == 0),
                        stop=False,
                    )
                nc.tensor.matmul(
                    v_ps[:, :], ones_1[:, :], bv_sb[:, :], start=False, stop=True
                )
                nc.scalar.copy(v_sb[:, t, :], v_ps[:, :])

            # ---- attention ----
            for j in range(4):  # query block of 512
                P = []
                for i in range(4 * j + 4):  # key tile of 128
                    sc_ps = ps_s.tile([128, 512], f32, tag="scps")
                    nc.tensor.matmul(
                        sc_ps[:, :],
                        kt_sb[:, 128 * i : 128 * (i + 1)],
                        qk_sb[0:64, 512 * j : 512 * (j + 1)],
                        start=True,
                        stop=True,
                    )
                    p = ppool.tile([128, 512], bf16, tag="p")
                    nc.scalar.activation(p[:, :], sc_ps[:, :], AF.Exp)
                    if i >= 4 * j:
                        nc.vector.tensor_tensor(
                            p[:, :], p[:, :], mask_sb[:, i - 4 * j, :], ALU.mult
                        )
                    P.append(p)
                for tq in range(4):  # query tile of 128 within the block
                    t = 4 * j + tq
                    o_ps = ps_o.tile([128, CPC], f32, tag="ops")
                    rs_ps = ps_r.tile([128, 1], f32, tag="rsps")
                    for i in range(t + 1):
                        lhsT = P[i][:, 128 * tq : 128 * (tq + 1)]
                        nc.tensor.matmul(
                            o_ps[:, :],
                            lhsT,
                            v_sb[:, i, :],
                            start=(i == 0),
                            stop=(i == t),
                        )
                        nc.tensor.matmul(
                            rs_ps[:, :],
                            lhsT,
                            ones_k[:, :],
                            start=(i == 0),
                            stop=(i == t),
                        )
                    rs_sb = small.tile([128, 1], f32, tag="rs")
                    nc.scalar.copy(rs_sb[:, :], rs_ps[:, :])
                    rcp = small.tile([128, 1], f32, tag="rcp")
                    nc.vector.reciprocal(rcp[:, :], rs_sb[:, :])
                    o_sb = opool.tile([128, CPC], f32, tag="osb")
                    nc.vector.tensor_scalar(
                        o_sb[:, :], o_ps[:, :], rcp[:, 0:1], None, ALU.mult
                    )
                    nc.sync.dma_start(out=out[t, :, :], in_=o_sb[:, :])

    nc.compile()
    return nc


def _get_nc():
    if "nc" not in _cache:
        _cache["nc"] = _build()
    return _cache["nc"]


def _prep_in_maps(x, WQ_w, WQ_b, WK_w, WK_b, WV_w, WV_b):
    bf = mybir.dt.np(bf16)
    wqk = np.concatenate([WQ_w, WK_w], axis=0)  # [128, D]
    wqkT = np.ascontiguousarray(wqk.T.reshape(NCHUNK, 128, 128)).astype(
        np.float32, copy=False
    )
    bqk = np.concatenate([WQ_b, WK_b]).reshape(128, 1).astype(np.float32, copy=False)

    # masks[m, kk, qq] = 1 if 128*m + kk <= qq else 0
    kk = np.arange(128)[:, None]
    qq = np.arange(512)[None, :]
    masks = np.stack(
        [(128 * m + kk <= qq) for m in range(4)], axis=0
    ).astype(bf)

    in_maps = []
    for core in range(N_CORES):
        b, half = core // 2, core % 2
        xTb = np.ascontiguousarray(x[b].T).reshape(NCHUNK, 128, S)
        wv_sl = WV_w[half * CPC : (half + 1) * CPC]  # [CPC, D]
        wvT = np.ascontiguousarray(wv_sl.T).reshape(NCHUNK, 128, CPC)
        bv = np.ascontiguousarray(
            WV_b[half * CPC : (half + 1) * CPC].reshape(1, CPC)
        )
        in_maps.append(
            {
                "xT": xTb.astype(np.float32, copy=False),
                "wqkT": wqkT,
                "bqk": bqk,
                "wvT": wvT.astype(np.float32, copy=False),
                "bv": bv.astype(np.float32, copy=False),
                "masks": masks,
                "ones1": np.ones((1, 128), np.float32),
            }
        )
    return in_maps


def _run(in_maps, trace=False, **kw):
    nc = _get_nc()
    return run_bass_kernel_spmd(
        nc, in_maps, core_ids=list(range(N_CORES)), trace=trace, **kw
    )


def kernel(x, WQ_w, WQ_b, WK_w, WK_b, WV_w, WV_b):
    x = np.asarray(x, dtype=np.float32)
    in_maps = _prep_in_maps(
        x,
        np.asarray(WQ_w, np.float32),
        np.asarray(WQ_b, np.float32),
        np.asarray(WK_w, np.float32),
        np.asarray(WK_b, np.float32),
        np.asarray(WV_w, np.float32),
        np.asarray(WV_b, np.float32),
    )
    res = _run(in_maps, trace=False)
    out = np.empty((B, S, D), dtype=np.float32)
    for core in range(N_CORES):
        b, half = core // 2, core % 2
        shard = res.results[core]["out"].reshape(S, CPC)
        out[b, :, half * CPC : (half + 1) * CPC] = shard
    return out
